# revision 1
# baseline (speedup 1.0000x reference)
"""Multi-head attention (B=2, N=2048, D=1024, H=16) on 8 Trainium2 cores.

Sharding: data-parallel over batch (2) x tensor-parallel over head groups (4).
Core c handles batch c//4, heads 4*(c%4) .. 4*(c%4)+3.

Per-core kernel (matmuls at full PE rate via float32r, P*V in bf16):
  front:   kT = ([Wk;bk]^T @ [x^T;1])   (channels on partitions)
           v  = ([x;1] @ [Wv;bv])       (tokens on partitions, [v|1] blocks)
  per query-tile of 512 (ACT-exp is the pacing engine, ~32us/tile):
           qT slice = ([Wq;bq]^T @ [x^T;1])
           for each key-ptile of 128:
             S^T[:,h,:] = kT_h^T qT_h    (keys on partitions, 4 single-shot
                                          matmuls into the 4 banks of one tile)
             P^T = exp(SCALE * S^T)      (one ACT op over all 4 heads)
             [O^T_h; sums_h] += [v_h|1]^T P^T_h   (per-head chain, own bank)
           O^T_h *= broadcast(1/sums_h)  (DVE recip + gpsimd bcast + DVE mul)
           out[tokens of this tile] = sum_h O^T_h^T @ Wo_h  (K=64 chains)
Host: out[b] = sum of the 4 group partials + b_o.
"""

import sys

sys.path.insert(0, "/opt/trn_rl_repo")

import numpy as np

B, N, D, H = 2, 2048, 1024, 16
SUB = D // H  # 64
GROUPS = 4  # tensor-parallel head groups
NH = H // GROUPS  # 4 local heads per core
CH = NH * SUB  # 256 local channels
NCORES = 8


def build_nc(NT=N, DK=D, DO=D, nh=NH, name="mha"):
    import concourse.mybir as mybir
    from concourse import bacc
    from concourse.tile import TileContext

    f32 = mybir.dt.float32
    f32r = mybir.dt.float32r
    bf16 = mybir.dt.bfloat16
    Exp = mybir.ActivationFunctionType.Exp
    mult = mybir.AluOpType.mult

    sub = 64
    ch = nh * sub
    KT = DK // 128  # contraction ptiles
    CHT = ch // 128  # channel ptiles
    TOKT = NT // 128  # token/key ptiles
    QT = NT // 512  # query tiles
    TPQ = TOKT // QT  # token ptiles emitted per query tile (4)
    scale = sub ** -0.5

    nc = bacc.Bacc(None, name=name)
    xT = nc.dram_tensor("xT", [DK, NT], f32r, kind="ExternalInput")
    wq = nc.dram_tensor("wq", [DK + 1, ch], f32r, kind="ExternalInput")
    wk = nc.dram_tensor("wk", [DK + 1, ch], f32r, kind="ExternalInput")
    wv = nc.dram_tensor("wv", [DK + 1, ch], f32r, kind="ExternalInput")
    wo = nc.dram_tensor("wo", [ch, DO], f32r, kind="ExternalInput")
    ones_in = nc.dram_tensor("ones", [1, 512], f32r, kind="ExternalInput")
    bq = nc.dram_tensor("bq", [ch, 1], f32, kind="ExternalInput")
    bk = nc.dram_tensor("bk", [ch, 1], f32, kind="ExternalInput")
    out = nc.dram_tensor("out", [NT, DO], f32, kind="ExternalOutput")

    with TileContext(nc) as tc:
        with tc.tile_pool(name="persist", bufs=1) as pp:
            ones = pp.tile([1, 512], f32r)
            qT_sb = pp.tile([128, CHT, NT], f32r)
            kT_sb = pp.tile([128, CHT, NT], f32r)
            v_sb = pp.tile([128, TOKT, ch], bf16)
            ones_c = pp.tile([128, 1], bf16)
            oT_sb = pp.tile([128, CHT, NT], f32r)
            wo_sb = pp.tile([128, CHT, DO], f32r)
            nc.sync.dma_start(ones[:], ones_in[:])
            ones_f = pp.tile([128, 1], f32)
            nc.vector.memset(ones_f[:], 1.0)
            nc.vector.tensor_copy(ones_c[:], ones_f[:])
            zeros_c = pp.tile([128, 128], bf16)
            zeros_f = pp.tile([128, 128], f32)
            nc.vector.memset(zeros_f[:], 0.0)
            nc.vector.tensor_copy(zeros_c[:], zeros_f[:])
            bqk_sb = pp.tile([128, 2, CHT], f32)
            for i, bsrc in enumerate((bq, bk)):
                for ct in range(CHT):
                    nc.sync.dma_start(
                        bqk_sb[:, i, ct : ct + 1], bsrc[ct * 128 : (ct + 1) * 128, :]
                    )
            for ct in range(CHT):
                nc.sync.dma_start(wo_sb[:, ct, :], wo[ct * 128 : (ct + 1) * 128, :])

            with tc.tile_pool(name="xp", bufs=1) as xp, \
                 tc.tile_pool(name="wp", bufs=1) as wp, \
                 tc.tile_pool(name="stp", bufs=2, space="PSUM") as stp, \
                 tc.tile_pool(name="acc", bufs=4, space="PSUM") as acc, \
                 tc.tile_pool(name="ptp", bufs=8) as ptp, \
                 tc.tile_pool(name="nrm", bufs=4) as nrm, \
                 tc.tile_pool(name="osg", bufs=4) as osg:
                xt = xp.tile([128, KT, NT], f32r)
                w_sb = {}

                def load_w(nm, dram):
                    wch = ch
                    wt = wp.tile([128, KT, wch], f32r, name=f"{nm}t", tag=f"{nm}t")
                    for kt in range(KT):
                        nc.sync.dma_start(
                            wt[:, kt, :], dram[kt * 128 : (kt + 1) * 128, :]
                        )
                    wb = wp.tile([1, wch], f32r, name=f"{nm}b", tag=f"{nm}b")
                    nc.sync.dma_start(wb[:], dram[DK : DK + 1, :])
                    w_sb[nm] = (wt, wb)

                load_w("wk", wk)
                for kt in range(KT):
                    nc.sync.dma_start(xt[:, kt, :], xT[kt * 128 : (kt + 1) * 128, :])
                load_w("wv", wv)
                load_w("wq", wq)

                def qk_proj(dst, nm, mt, qt, pool=None, tag="acc"):
                    """dst[:, mt, qt*512:+512] = (W^T @ x^T) slice + per-partition bias."""
                    wt, wb = w_sb[nm]
                    ps = (pool or acc).tile([128, 512], f32, name="ps", tag=tag)
                    for kt in range(KT):
                        nc.tensor.matmul(
                            ps[:],
                            lhsT=wt[:, kt, mt * 128 : (mt + 1) * 128],
                            rhs=xt[:, kt, qt * 512 : (qt + 1) * 512],
                            start=(kt == 0),
                            stop=(kt == KT - 1),
                        )
                    nc.vector.tensor_scalar_add(
                        dst[:, mt, qt * 512 : (qt + 1) * 512],
                        ps[:],
                        bqk_sb[:, 0 if nm == "wq" else 1, mt : mt + 1],
                    )

                def v_proj(tt, pool=None, tag="acc"):
                    """v_sb[:, tt, :] = ([x;1] @ [Wv;bv])."""
                    wt, wb = w_sb["wv"]
                    ps = (pool or acc).tile([128, ch], f32, name="psv", tag=tag)
                    for kt in range(KT):
                        nc.tensor.matmul(
                            ps[:],
                            lhsT=xt[:, kt, tt * 128 : (tt + 1) * 128],
                            rhs=wt[:, kt, :],
                            start=(kt == 0),
                            stop=False,
                        )
                    nc.tensor.matmul(
                        ps[:],
                        lhsT=ones[0:1, 0:128],
                        rhs=wb[:],
                        start=False,
                        stop=True,
                    )
                    nc.vector.tensor_copy(v_sb[:, tt, :], ps[:])

                def outproj_piece(tt, nt):
                    ps = acc.tile([128, 512], f32, name="ops", tag="acc")
                    for ct in range(CHT):
                        nc.tensor.matmul(
                            ps[:],
                            lhsT=oT_sb[:, ct, tt * 128 : (tt + 1) * 128],
                            rhs=wo_sb[:, ct, nt * 512 : (nt + 1) * 512],
                            start=(ct == 0),
                            stop=(ct == CHT - 1),
                        )
                    stg = osg.tile([128, 512], f32, name="stg", tag="stg")
                    nc.vector.tensor_copy(stg[:], ps[:])
                    nc.sync.dma_start(
                        out[tt * 128 : (tt + 1) * 128, nt * 512 : (nt + 1) * 512],
                        stg[:],
                    )
                # streamed projections: minimal front, everything else
                # trickles through the spare acc slot under the ACT-paced loop
                from collections import deque

                NVF = 2  # v tiles projected upfront; the rest stream just-in-time
                pending = deque()
                for tt in range(NVF, TOKT):
                    pending.append(("v", tt))
                for qt in range(1, QT):
                    for mt in range(CHT):
                        pending.append(("q", mt, qt))

                def emit(item):
                    kind = item[0]
                    if kind == "v":
                        v_proj(item[1])
                    elif kind == "q":
                        qk_proj(qT_sb, "wq", item[1], item[2])
                    elif kind == "o":
                        outproj_piece(item[1], item[2])

                # minimal front: all of kT, first two v tiles, qT of qt 0.
                # chains alternate between the acc slots and the (idle) stp
                # slots so six are in flight instead of four
                front = [("k", mt, qt) for mt in range(CHT) for qt in range(QT)]
                front += [("vf", tt) for tt in range(NVF)]
                front += [("qf", mt) for mt in range(CHT)]
                for i, item in enumerate(front):
                    pool, tag = (stp, "st") if i % 2 else (acc, "acc")
                    if item[0] == "k":
                        qk_proj(kT_sb, "wk", item[1], item[2], pool=pool, tag=tag)
                    elif item[0] == "vf":
                        v_proj(item[1], pool=pool, tag=tag)
                    else:
                        qk_proj(qT_sb, "wq", item[1], 0, pool=pool, tag=tag)
                for qt in range(QT):
                    ot = [
                        acc.tile([128, 512], f32, name=f"otp{p}", tag="acc")
                        for p in range(nh // 2)
                    ]
                    sm = acc.tile([97, 512], f32, name="sm", tag="acc")
                    for kt2 in range(TOKT):
                        if pending and (pending[0][0] == "v" or kt2 % 2 == 0):
                            emit(pending.popleft())
                        first, last = kt2 == 0, kt2 == TOKT - 1
                        # two half-tiles (2 heads / 2 banks each), double-buffered:
                        # exp of one half pipelines against S-matmuls of the other
                        for half in range(nh // 2):
                            st = stp.tile([128, 2, 512], f32, name="st", tag="st")
                            for hh in range(2):
                                h = 2 * half + hh
                                bp = 64 * hh
                                nc.tensor.matmul(
                                    st[:, hh, :],
                                    lhsT=kT_sb[bp : bp + 64, half, kt2 * 128 : (kt2 + 1) * 128],
                                    rhs=qT_sb[bp : bp + 64, half, qt * 512 : (qt + 1) * 512],
                                    start=True,
                                    stop=True,
                                )
                            pt = ptp.tile([128, 2, 512], bf16, name="pt", tag="pt")
                            nc.scalar.activation(pt[:], st[:], Exp, scale=scale)
                            if first and half == 0:
                                # open the shared-bank has_written groups with
                                # zero matmuls, emitted after the first S/exp so
                                # they don't head-of-line-block the PE stream on
                                # the previous qt's normalize
                                for pp_ in range(nh // 2):
                                    nc.tensor.matmul(
                                        ot[pp_][:], lhsT=zeros_c[:], rhs=v_sb[:, 0:2, :],
                                        start=True, stop=False, skip_group_check=True,
                                    )
                                nc.tensor.matmul(
                                    sm[:], lhsT=zeros_c[:, 0:97], rhs=v_sb[:, 0:2, :],
                                    start=True, stop=False, skip_group_check=True,
                                )
                            for hh in range(2):
                                h = 2 * half + hh
                                nc.tensor.matmul(
                                    ot[half][64 * hh : 64 * hh + 64, :],
                                    lhsT=v_sb[:, kt2, 64 * h : 64 * h + 64],
                                    rhs=pt[:, hh, :],
                                    start=False,
                                    stop=last,
                                    skip_group_check=True,
                                )
                                nc.tensor.matmul(
                                    sm[32 * h : 32 * h + 1, :],
                                    lhsT=ones_c[:],
                                    rhs=pt[:, hh, :],
                                    start=False,
                                    stop=last,
                                    tile_position=(0, 32 * h),
                                    skip_group_check=True,
                                )
                    for h in range(nh):
                        bp = 64 * (h % 2)
                        rcp = nrm.tile([97, 512], f32, name="rcp", tag="rcp")
                        row0 = nrm.tile([1, 512], f32, name="row0", tag="row0")
                        bc = nrm.tile([64, 512], f32, name="bc", tag="bc")
                        nc.vector.reciprocal(rcp[32 * h : 32 * h + 1, :], sm[32 * h : 32 * h + 1, :])
                        # gpsimd broadcast reads physical partition 0: stage there
                        nc.sync.dma_start(row0[:], rcp[32 * h : 32 * h + 1, :])
                        nc.gpsimd.partition_broadcast(bc[:], row0[:], channels=64)
                        nc.vector.tensor_tensor(
                            out=oT_sb[bp : bp + 64, h // 2, qt * 512 : (qt + 1) * 512],
                            in0=ot[h // 2][bp : bp + 64, :],
                            in1=bc[:],
                            op=mult,
                        )
                    for tt in range(qt * TPQ, min((qt + 1) * TPQ, TOKT)):
                        for nt in range(DO // 512):
                            pending.append(("o", tt, nt))
                while pending:
                    emit(pending.popleft())
    nc.finalize()
    return nc


def make_in_maps(x, W_qkv, b_qkv, W_o):
    """Shard full inputs into per-core input maps (core c: batch c//4, group c%4)."""
    x = np.asarray(x, dtype=np.float32)
    W_qkv = np.asarray(W_qkv, dtype=np.float32)
    b_qkv = np.asarray(b_qkv, dtype=np.float32)
    W_o = np.asarray(W_o, dtype=np.float32)
    in_maps = []
    for c in range(NCORES):
        b, g = divmod(c, GROUPS)
        cols = slice(CH * g, CH * (g + 1))
        m = {
            "xT": np.ascontiguousarray(x[b].T),
            "wq": np.ascontiguousarray(
                np.concatenate([W_qkv[:, 0 * D : 1 * D][:, cols], b_qkv[0 * D : 1 * D][cols][None, :]], 0)
            ),
            "wk": np.ascontiguousarray(
                np.concatenate([W_qkv[:, 1 * D : 2 * D][:, cols], b_qkv[1 * D : 2 * D][cols][None, :]], 0)
            ),
            "wv": np.ascontiguousarray(
                np.concatenate([W_qkv[:, 2 * D : 3 * D][:, cols], b_qkv[2 * D : 3 * D][cols][None, :]], 0)
            ),
            "wo": np.ascontiguousarray(W_o[cols, :]),
            "ones": np.ones((1, 512), dtype=np.float32),
            "bq": np.ascontiguousarray(b_qkv[0 * D : 1 * D][cols][:, None]),
            "bk": np.ascontiguousarray(b_qkv[1 * D : 2 * D][cols][:, None]),
        }
        in_maps.append(m)
    return in_maps


_NC = None


def get_nc():
    global _NC
    if _NC is None:
        _NC = build_nc()
    return _NC


def kernel(x, W_qkv, b_qkv, W_o, b_o):
    from concourse import bass_utils

    b_o = np.asarray(b_o, dtype=np.float32)
    in_maps = make_in_maps(x, W_qkv, b_qkv, W_o)
    res = bass_utils.run_bass_kernel_spmd(get_nc(), in_maps, core_ids=list(range(NCORES)))
    out = np.empty((B, N, D), dtype=np.float32)
    for b in range(B):
        acc = res.results[4 * b]["out"].copy()
        for g in range(1, GROUPS):
            acc += res.results[4 * b + g]["out"]
        out[b] = acc + b_o
    return out



# revision 8
# speedup vs baseline: 1.3330x; 1.3330x over previous
"""Multi-head attention (B=2, N=2048, D=1024, H=16) on 8 Trainium2 cores.

Sharding: data-parallel over batch (2) x tensor-parallel over head groups (4).
Core c handles batch c//4, heads 4*(c%4) .. 4*(c%4)+3.

Per-core kernel (ACT exp is the long pole; PE kept just under it):
  - qkv projections in f32r (full PE rate at >=256 free).
  - v stored as [v|1] bf16 (ones column from the bias row), so PV
    accumulates softmax sums in column 64 for free.
  - S^T = kT^T qT in f32r, exp on ACT -> P^T in bf16.
  - PV "flipped": P^T tiles are the stationary operand, [v|1] (65 cols)
    is the moving one -> 65 cycles per (key-tile, query-ptile) instead of
    512, output O[q, 65] with queries on partitions.
  - normalize on DVE with per-partition scalars (1/sums), then PE
    transpose (via identity permutation) back to O^T for the out-proj.
  - out = O^T^T Wo in bf16, K=64 chains per 128-channel block.
Host: out[b] = sum of the 4 group partials + b_o.
"""

import sys

sys.path.insert(0, "/opt/trn_rl_repo")

import numpy as np
import ml_dtypes

B, N, D, H = 2, 2048, 1024, 16
SUB = D // H  # 64
GROUPS = 4  # tensor-parallel head groups
NH = H // GROUPS  # 4 local heads per core
CH = NH * SUB  # 256 local channels
NCORES = 8
VW = SUB + 1  # 65: per-head [v|1] width


def build_nc(NT=N, DK=D, DO=D, nh=NH, name="mha"):
    import concourse.mybir as mybir
    from concourse import bacc
    from concourse.tile import TileContext

    f32 = mybir.dt.float32
    f32r = mybir.dt.float32r
    bf16 = mybir.dt.bfloat16
    Exp = mybir.ActivationFunctionType.Exp
    mult = mybir.AluOpType.mult

    sub = 64
    ch = nh * sub  # 256
    KT = DK // 128  # 8 contraction ptiles
    CHT = ch // 128  # 2 channel ptiles
    TOKT = NT // 128  # 16 token/key ptiles
    QT = NT // 512  # 4 query tiles
    scale = sub ** -0.5

    nc = bacc.Bacc(None, name=name)
    xTd = nc.dram_tensor("xT", [128, KT, NT], f32r, kind="ExternalInput")
    wqd = nc.dram_tensor("wq", [128, KT, ch], f32r, kind="ExternalInput")
    wkd = nc.dram_tensor("wk", [128, KT, ch], f32r, kind="ExternalInput")
    wvd = nc.dram_tensor("wv", [128, KT, nh * VW], f32r, kind="ExternalInput")
    wvbd = nc.dram_tensor("wvb", [1, nh * VW], f32r, kind="ExternalInput")
    onesd = nc.dram_tensor("ones", [1, 128], f32r, kind="ExternalInput")
    identd = nc.dram_tensor("ident", [128, 128], bf16, kind="ExternalInput")
    wod = nc.dram_tensor("wo", [ch, DO], bf16, kind="ExternalInput")
    bqd = nc.dram_tensor("bq", [ch, 1], f32, kind="ExternalInput")
    bkd = nc.dram_tensor("bk", [ch, 1], f32, kind="ExternalInput")
    out = nc.dram_tensor("out", [NT, DO], f32, kind="ExternalOutput")

    with TileContext(nc) as tc:
        with tc.tile_pool(name="persist", bufs=1) as pp:
            xT = pp.tile([128, KT, NT], f32r)
            wq_sb = pp.tile([128, KT, ch], f32r)
            wk_sb = pp.tile([128, KT, ch], f32r)
            wv_sb = pp.tile([128, KT, nh * VW], f32r)
            wvb_sb = pp.tile([1, nh * VW], f32r)
            ones_sb = pp.tile([1, 128], f32r)
            ident = pp.tile([128, 128], bf16)
            qT_sb = pp.tile([128, CHT, NT], f32r)
            kT_sb = pp.tile([128, CHT, NT], f32r)
            v1 = pp.tile([128, TOKT, nh * VW], bf16)
            oT_sb = pp.tile([128, CHT, NT], bf16)
            wo_sb = pp.tile([128, CHT, DO], bf16)
            bqk_sb = pp.tile([128, 2, CHT], f32)
            zeros16 = pp.tile([128, 128], bf16)
            dumm16 = pp.tile([128, nh * VW], bf16)

            nc.sync.dma_start(wk_sb[:], wkd[:])
            nc.sync.dma_start(xT[:], xTd[:])
            nc.sync.dma_start(wq_sb[:], wqd[:])
            nc.sync.dma_start(wv_sb[:], wvd[:])
            nc.sync.dma_start(wvb_sb[:], wvbd[:])
            nc.sync.dma_start(ones_sb[:], onesd[:])
            nc.sync.dma_start(ident[:], identd[:])
            for ct in range(CHT):
                nc.sync.dma_start(wo_sb[:, ct, :], wod[ct * 128 : (ct + 1) * 128, :])
            for i, bsrc in enumerate((bqd, bkd)):
                for ct in range(CHT):
                    nc.sync.dma_start(
                        bqk_sb[:, i, ct : ct + 1], bsrc[ct * 128 : (ct + 1) * 128, :]
                    )
            zf = pp.tile([128, 128], f32)
            nc.vector.memset(zf[:], 0.0)
            nc.vector.tensor_copy(zeros16[:], zf[:])
            nc.vector.memset(dumm16[:], 0.0)

            with tc.tile_pool(name="stp", bufs=2, space="PSUM") as stp, \
                 tc.tile_pool(name="accp", bufs=2, space="PSUM") as accp, \
                 tc.tile_pool(name="prj", bufs=2, space="PSUM") as prj, \
                 tc.tile_pool(name="ptp", bufs=4) as ptp, \
                 tc.tile_pool(name="nrm", bufs=3) as nrm, \
                 tc.tile_pool(name="osg", bufs=4) as osg:

                def qk_chain(nm, mt, ts):
                    w = wq_sb if nm == "q" else wk_sb
                    dst = qT_sb if nm == "q" else kT_sb
                    ps = prj.tile([128, 512], f32, name="ps", tag="prj")
                    for kt in range(KT):
                        nc.tensor.matmul(
                            ps[:],
                            lhsT=w[:, kt, mt * 128 : (mt + 1) * 128],
                            rhs=xT[:, kt, ts * 512 : (ts + 1) * 512],
                            start=(kt == 0),
                            stop=(kt == KT - 1),
                        )
                    nc.vector.tensor_scalar_add(
                        dst[:, mt, ts * 512 : (ts + 1) * 512],
                        ps[:],
                        bqk_sb[:, 0 if nm == "q" else 1, mt : mt + 1],
                    )

                def v_chain(tt):
                    ps = prj.tile([128, nh * VW], f32, name="psv", tag="prj")
                    for kt in range(KT):
                        nc.tensor.matmul(
                            ps[:],
                            lhsT=xT[:, kt, tt * 128 : (tt + 1) * 128],
                            rhs=wv_sb[:, kt, :],
                            start=(kt == 0),
                            stop=False,
                        )
                    nc.tensor.matmul(
                        ps[:], lhsT=ones_sb[:], rhs=wvb_sb[:], start=False, stop=True
                    )
                    nc.vector.tensor_copy(v1[:, tt, :], ps[:])

                def o_chain(tt, nt):
                    ps = prj.tile([128, 512], f32, name="pso", tag="prj")
                    for ct in range(CHT):
                        nc.tensor.matmul(
                            ps[:],
                            lhsT=oT_sb[:, ct, tt * 128 : (tt + 1) * 128],
                            rhs=wo_sb[:, ct, nt * 512 : (nt + 1) * 512],
                            start=(ct == 0),
                            stop=(ct == CHT - 1),
                        )
                    stg = osg.tile([128, 512], f32, name="stg", tag="stg")
                    nc.vector.tensor_copy(stg[:], ps[:])
                    nc.sync.dma_start(
                        out[tt * 128 : (tt + 1) * 128, nt * 512 : (nt + 1) * 512],
                        stg[:],
                    )

                def t_piece(o16, h, qt):
                    """O[q,64] -> O^T[64,q] via PE transpose, land in oT_sb."""
                    mt = h // 2
                    bp = 64 * (h % 2)
                    for qi in range(4):
                        tp = prj.tile([64, 128], bf16, name="tp", tag="prj")
                        nc.tensor.matmul(
                            tp[:],
                            lhsT=o16[:, qi, :],
                            rhs=ident[:],
                            is_transpose=True,
                        )
                        nc.vector.tensor_copy(
                            oT_sb[
                                bp : bp + 64,
                                mt,
                                qt * 512 + qi * 128 : qt * 512 + (qi + 1) * 128,
                            ],
                            tp[:],
                        )

                done = set()
                from collections import deque

                pending = deque()

                def need(kind, *a):
                    if (kind,) + a in done:
                        return
                    done.add((kind,) + a)
                    if kind == "q" or kind == "k":
                        qk_chain(kind, *a)
                    elif kind == "v":
                        v_chain(*a)

                def emit(item):
                    if item[0] in ("q", "k", "v"):
                        need(*item)
                    elif item[0] == "t":
                        t_piece(item[1], item[2], item[3])
                    else:
                        o_chain(item[1], item[2])

                for ts in range(QT):
                    pending.append(("k", 0, ts))
                pending.append(("q", 0, 0))
                for tt in range(TOKT // 2):
                    pending.append(("v", tt))
                for ts in range(QT):
                    pending.append(("k", 1, ts))
                pending.append(("q", 1, 0))
                for tt in range(TOKT // 2, TOKT):
                    pending.append(("v", tt))

                for qt in range(QT):
                    for h in range(nh):
                        mt = h // 2
                        bp = 64 * (h % 2)
                        need("k", mt, 0)
                        need("q", mt, qt)
                        o_acc = accp.tile([128, 4, VW], f32, name="oacc", tag="acc")
                        nc.tensor.matmul(
                            o_acc[:],
                            lhsT=zeros16[:],
                            rhs=dumm16[:],
                            start=True,
                            stop=False,
                            skip_group_check=True,
                        )
                        for m in range(TOKT // 2):
                            need("k", mt, m // 2)
                            need("v", 2 * m)
                            need("v", 2 * m + 1)
                            if pending:
                                emit(pending.popleft())
                            st = stp.tile([128, 2, 512], f32, name="st", tag="st")
                            for j in range(2):
                                nc.tensor.matmul(
                                    st[:, j, :],
                                    lhsT=kT_sb[
                                        bp : bp + 64,
                                        mt,
                                        (2 * m + j) * 128 : (2 * m + j + 1) * 128,
                                    ],
                                    rhs=qT_sb[
                                        bp : bp + 64, mt, qt * 512 : (qt + 1) * 512
                                    ],
                                    start=True,
                                    stop=True,
                                )
                            pt = ptp.tile([128, 2, 512], bf16, name="pt", tag="pt")
                            nc.scalar.activation(pt[:], st[:], Exp, scale=scale)
                            last = m == TOKT // 2 - 1
                            for j in range(2):
                                for qi in range(4):
                                    nc.tensor.matmul(
                                        o_acc[:, qi, :],
                                        lhsT=pt[
                                            :, j, qi * 128 : (qi + 1) * 128
                                        ],
                                        rhs=v1[
                                            :, 2 * m + j, VW * h : VW * h + VW
                                        ],
                                        start=False,
                                        stop=(last and j == 1),
                                        skip_group_check=True,
                                    )
                        # normalize on DVE: per-query 1/sums is a per-partition
                        # scalar in this layout
                        rcp = nrm.tile([128, 4, 1], f32, name="rcp", tag="rcp")
                        o16 = nrm.tile([128, 4, 64], bf16, name="o16", tag="o16")
                        nc.vector.reciprocal(rcp[:], o_acc[:, :, 64:65])
                        for qi in range(4):
                            nc.vector.tensor_scalar_mul(
                                o16[:, qi, :],
                                o_acc[:, qi, 0:64],
                                rcp[:, qi, 0:1],
                            )
                        pending.append(("t", o16, h, qt))
                    for tt in range(qt * (TOKT // QT), (qt + 1) * (TOKT // QT)):
                        for nt in range(DO // 512):
                            pending.append(("o", tt, nt))
                while pending:
                    emit(pending.popleft())
    nc.finalize()
    return nc


def make_in_maps(x, W_qkv, b_qkv, W_o):
    """Shard full inputs into per-core input maps (core c: batch c//4, group c%4)."""
    x = np.asarray(x, dtype=np.float32)
    W_qkv = np.asarray(W_qkv, dtype=np.float32)
    b_qkv = np.asarray(b_qkv, dtype=np.float32)
    W_o = np.asarray(W_o, dtype=np.float32)
    bf16 = ml_dtypes.bfloat16
    KT = D // 128

    def fold(a):  # [D, C] -> [128, KT, C]
        return np.ascontiguousarray(a.reshape(KT, 128, -1).transpose(1, 0, 2))

    in_maps = []
    for c in range(NCORES):
        b, g = divmod(c, GROUPS)
        cols = slice(CH * g, CH * (g + 1))
        Wv = W_qkv[:, 2 * D : 3 * D][:, cols]
        bv = b_qkv[2 * D : 3 * D][cols]
        Wv_pad = np.zeros((D, NH * VW), dtype=np.float32)
        bv_pad = np.zeros((NH * VW,), dtype=np.float32)
        for h in range(NH):
            Wv_pad[:, VW * h : VW * h + SUB] = Wv[:, SUB * h : SUB * (h + 1)]
            bv_pad[VW * h : VW * h + SUB] = bv[SUB * h : SUB * (h + 1)]
            bv_pad[VW * h + SUB] = 1.0
        m = {
            "xT": fold(x[b].T),
            "wq": fold(W_qkv[:, 0 * D : 1 * D][:, cols]),
            "wk": fold(W_qkv[:, 1 * D : 2 * D][:, cols]),
            "wv": fold(Wv_pad),
            "wvb": bv_pad[None, :].copy(),
            "ones": np.ones((1, 128), dtype=np.float32),
            "ident": np.eye(128, dtype=np.float32).astype(bf16),
            "wo": np.ascontiguousarray(W_o[cols, :]).astype(bf16),
            "bq": np.ascontiguousarray(b_qkv[0 * D : 1 * D][cols][:, None]),
            "bk": np.ascontiguousarray(b_qkv[1 * D : 2 * D][cols][:, None]),
        }
        in_maps.append(m)
    return in_maps


_NC = None


def get_nc():
    global _NC
    if _NC is None:
        _NC = build_nc()
    return _NC


def kernel(x, W_qkv, b_qkv, W_o, b_o):
    from concourse import bass_utils

    b_o = np.asarray(b_o, dtype=np.float32)
    in_maps = make_in_maps(x, W_qkv, b_qkv, W_o)
    res = bass_utils.run_bass_kernel_spmd(get_nc(), in_maps, core_ids=list(range(NCORES)))
    out = np.empty((B, N, D), dtype=np.float32)
    for b in range(B):
        acc = res.results[4 * b]["out"].copy()
        for g in range(1, GROUPS):
            acc += res.results[4 * b + g]["out"]
        out[b] = acc + b_o
    return out


# revision 11
# speedup vs baseline: 1.4374x; 1.0783x over previous
"""Multi-head attention (B=2, N=2048, D=1024, H=16) on 8 Trainium2 cores.

Sharding: data-parallel over batch (2) x tensor-parallel over head groups (4).
Core c handles batch c//4, heads 4*(c%4) .. 4*(c%4)+3.

Per-core kernel (ACT exp is the long pole; PE kept just under it):
  - x and all projection weights stream in as bf16 (x split per k-tile so
    the first chains start ~3us in); projections matmul in bf16 at full
    rate, accumulate f32, drain to f32r q/k with the bias added on DVE.
  - v stored as [v|1] bf16 (ones column via the bias row), so PV
    accumulates softmax sums in column 64 for free.
  - S^T = kT^T qT in f32r, exp on ACT -> P^T bf16.
  - PV "flipped": P^T tiles are the stationary operand, [v|1] (65 cols)
    the moving one -> 65 cycles per (key-tile, query-ptile) instead of
    512; O lands [query, 65] with queries on partitions.
  - normalize on DVE with per-partition 1/sums, then one XBAR DMA
    transpose per (head, query-tile) puts O^T into SBUF for the out-proj.
  - out = O^T^T Wo in bf16, K=64 chains per 128-channel block.
Host: out[b] = sum of the 4 group partials + b_o.
"""

import sys

sys.path.insert(0, "/opt/trn_rl_repo")

import numpy as np
import ml_dtypes

B, N, D, H = 2, 2048, 1024, 16
SUB = D // H  # 64
GROUPS = 4  # tensor-parallel head groups
NH = H // GROUPS  # 4 local heads per core
CH = NH * SUB  # 256 local channels
NCORES = 8
VW = SUB + 1  # 65: per-head [v|1] width


def build_nc(NT=N, DK=D, DO=D, nh=NH, name="mha"):
    import concourse.mybir as mybir
    from concourse import bacc
    from concourse.tile import TileContext

    f32 = mybir.dt.float32
    f32r = mybir.dt.float32r
    bf16 = mybir.dt.bfloat16
    Exp = mybir.ActivationFunctionType.Exp

    sub = 64
    ch = nh * sub  # 256
    KT = DK // 128  # 8 contraction ptiles
    CHT = ch // 128  # 2 channel ptiles
    TOKT = NT // 128  # 16 token/key ptiles
    QT = NT // 512  # 4 query tiles
    scale = sub ** -0.5

    nc = bacc.Bacc(None, name=name)
    xTd = nc.dram_tensor("xT", [128, KT, NT], bf16, kind="ExternalInput")
    wqd = nc.dram_tensor("wq", [128, KT, ch], bf16, kind="ExternalInput")
    wkd = nc.dram_tensor("wk", [128, KT, ch], bf16, kind="ExternalInput")
    wvd = nc.dram_tensor("wv", [128, KT, nh * VW], bf16, kind="ExternalInput")
    wvbd = nc.dram_tensor("wvb", [1, nh * VW], bf16, kind="ExternalInput")
    onesd = nc.dram_tensor("ones", [1, 128], bf16, kind="ExternalInput")
    wod = nc.dram_tensor("wo", [ch, DO], bf16, kind="ExternalInput")
    bqd = nc.dram_tensor("bq", [ch, 1], f32, kind="ExternalInput")
    bkd = nc.dram_tensor("bk", [ch, 1], f32, kind="ExternalInput")
    out = nc.dram_tensor("out", [NT, DO], f32, kind="ExternalOutput")

    with TileContext(nc) as tc:
        with tc.tile_pool(name="persist", bufs=1) as pp:
            xT = pp.tile([128, KT, NT], bf16)
            wq_sb = pp.tile([128, KT, ch], bf16)
            wk_sb = pp.tile([128, KT, ch], bf16)
            wv_sb = pp.tile([128, KT, nh * VW], bf16)
            wvb_sb = pp.tile([1, nh * VW], bf16)
            ones_sb = pp.tile([1, 128], bf16)
            qT_sb = pp.tile([128, CHT, NT], f32r)
            kT_sb = pp.tile([128, CHT, NT], f32r)
            v1 = pp.tile([128, TOKT, nh * VW], bf16)
            # O^T staging: token t of qt decomposes as (qt, qi, p)
            oT_sb = pp.tile([128, CHT, QT, 4, 128], bf16)
            wo_sb = pp.tile([128, CHT, DO], bf16)
            bqk_sb = pp.tile([128, 2, CHT], f32)
            zeros16 = pp.tile([128, 128], bf16)
            dumm16 = pp.tile([128, nh * VW], bf16)

            nc.sync.dma_start(wk_sb[:], wkd[:])
            nc.sync.dma_start(xT[:, 0, :], xTd[:, 0, :])
            nc.sync.dma_start(wq_sb[:], wqd[:])
            for kt in range(1, KT):
                nc.sync.dma_start(xT[:, kt, :], xTd[:, kt, :])
            nc.sync.dma_start(wv_sb[:], wvd[:])
            nc.sync.dma_start(wvb_sb[:], wvbd[:])
            nc.sync.dma_start(ones_sb[:], onesd[:])
            for ct in range(CHT):
                nc.sync.dma_start(wo_sb[:, ct, :], wod[ct * 128 : (ct + 1) * 128, :])
            for i, bsrc in enumerate((bqd, bkd)):
                for ct in range(CHT):
                    nc.sync.dma_start(
                        bqk_sb[:, i, ct : ct + 1], bsrc[ct * 128 : (ct + 1) * 128, :]
                    )
            zf = pp.tile([128, 128], f32)
            nc.vector.memset(zf[:], 0.0)
            nc.vector.tensor_copy(zeros16[:], zf[:])
            nc.vector.memset(dumm16[:], 0.0)

            with tc.tile_pool(name="stp", bufs=2, space="PSUM") as stp, \
                 tc.tile_pool(name="accp", bufs=2, space="PSUM") as accp, \
                 tc.tile_pool(name="prj", bufs=2, space="PSUM") as prj, \
                 tc.tile_pool(name="ptp", bufs=4) as ptp, \
                 tc.tile_pool(name="nrm", bufs=3) as nrm, \
                 tc.tile_pool(name="osg", bufs=4) as osg:

                def qk_chain(nm, mt, ts):
                    w = wq_sb if nm == "q" else wk_sb
                    dst = qT_sb if nm == "q" else kT_sb
                    ps = prj.tile([128, 512], f32, name="ps", tag="prj")
                    for kt in range(KT):
                        nc.tensor.matmul(
                            ps[:],
                            lhsT=w[:, kt, mt * 128 : (mt + 1) * 128],
                            rhs=xT[:, kt, ts * 512 : (ts + 1) * 512],
                            start=(kt == 0),
                            stop=(kt == KT - 1),
                        )
                    nc.vector.tensor_scalar_add(
                        dst[:, mt, ts * 512 : (ts + 1) * 512],
                        ps[:],
                        bqk_sb[:, 0 if nm == "q" else 1, mt : mt + 1],
                    )

                def v_chain(tt):
                    ps = prj.tile([128, nh * VW], f32, name="psv", tag="prj")
                    for kt in range(KT):
                        nc.tensor.matmul(
                            ps[:],
                            lhsT=xT[:, kt, tt * 128 : (tt + 1) * 128],
                            rhs=wv_sb[:, kt, :],
                            start=(kt == 0),
                            stop=False,
                        )
                    nc.tensor.matmul(
                        ps[:], lhsT=ones_sb[:], rhs=wvb_sb[:], start=False, stop=True
                    )
                    nc.vector.tensor_copy(v1[:, tt, :], ps[:])

                def o_chain(tt, nt):
                    ps = prj.tile([128, 512], f32, name="pso", tag="prj")
                    for ct in range(CHT):
                        nc.tensor.matmul(
                            ps[:],
                            lhsT=oT_sb[:, ct, tt // 4, tt % 4, :],
                            rhs=wo_sb[:, ct, nt * 512 : (nt + 1) * 512],
                            start=(ct == 0),
                            stop=(ct == CHT - 1),
                        )
                    stg = osg.tile([128, 512], f32, name="stg", tag="stg")
                    nc.vector.tensor_copy(stg[:], ps[:])
                    nc.sync.dma_start(
                        out[tt * 128 : (tt + 1) * 128, nt * 512 : (nt + 1) * 512],
                        stg[:],
                    )

                done = set()
                from collections import deque

                pending = deque()

                def need(kind, *a):
                    if (kind,) + a in done:
                        return
                    done.add((kind,) + a)
                    if kind == "q" or kind == "k":
                        qk_chain(kind, *a)
                    elif kind == "v":
                        v_chain(*a)

                def emit(item):
                    if item[0] in ("q", "k", "v"):
                        need(*item)
                    else:
                        o_chain(item[1], item[2])

                for ts in range(QT):
                    pending.append(("k", 0, ts))
                pending.append(("q", 0, 0))
                for tt in range(TOKT // 2):
                    pending.append(("v", tt))
                for ts in range(QT):
                    pending.append(("k", 1, ts))
                pending.append(("q", 1, 0))
                for tt in range(TOKT // 2, TOKT):
                    pending.append(("v", tt))

                for qt in range(QT):
                    o16pair = None
                    for h in range(nh):
                        mt = h // 2
                        bp = 64 * (h % 2)
                        need("k", mt, 0)
                        need("q", mt, qt)
                        o_acc = accp.tile([128, 4, VW], f32, name="oacc", tag="acc")
                        nc.tensor.matmul(
                            o_acc[:],
                            lhsT=zeros16[:],
                            rhs=dumm16[:],
                            start=True,
                            stop=False,
                            skip_group_check=True,
                        )
                        for m in range(TOKT // 2):
                            need("k", mt, m // 2)
                            need("v", 2 * m)
                            need("v", 2 * m + 1)
                            if pending:
                                emit(pending.popleft())
                            st = stp.tile([128, 2, 512], f32, name="st", tag="st")
                            for j in range(2):
                                nc.tensor.matmul(
                                    st[:, j, :],
                                    lhsT=kT_sb[
                                        bp : bp + 64,
                                        mt,
                                        (2 * m + j) * 128 : (2 * m + j + 1) * 128,
                                    ],
                                    rhs=qT_sb[
                                        bp : bp + 64, mt, qt * 512 : (qt + 1) * 512
                                    ],
                                    start=True,
                                    stop=True,
                                )
                            pt = ptp.tile([128, 2, 512], bf16, name="pt", tag="pt")
                            nc.scalar.activation(pt[:], st[:], Exp, scale=scale)
                            last = m == TOKT // 2 - 1
                            for j in range(2):
                                for qi in range(4):
                                    nc.tensor.matmul(
                                        o_acc[:, qi, :],
                                        lhsT=pt[:, j, qi * 128 : (qi + 1) * 128],
                                        rhs=v1[:, 2 * m + j, VW * h : VW * h + VW],
                                        start=False,
                                        stop=(last and j == 1),
                                        skip_group_check=True,
                                    )
                        # normalize on DVE (1/sums is per-partition here); each
                        # head pair shares a staging tile, then XBAR DMA
                        # transposes (full 128x128) land O^T in SBUF
                        if h % 2 == 0:
                            o16pair = nrm.tile(
                                [128, 4, 128], bf16, name="o16", tag="o16"
                            )
                        rcp = nrm.tile([128, 4, 1], f32, name="rcp", tag="rcp")
                        nc.vector.reciprocal(rcp[:], o_acc[:, :, 64:65])
                        for qi in range(4):
                            nc.vector.tensor_scalar_mul(
                                o16pair[:, qi, bp : bp + 64],
                                o_acc[:, qi, 0:64],
                                rcp[:, qi, 0:1],
                            )
                        if h % 2 == 1:
                            for qi in range(4):
                                nc.sync.dma_start_transpose(
                                    oT_sb[:, mt, qt, qi, :], o16pair[:, qi, :]
                                )
                    for tt in range(qt * (TOKT // QT), (qt + 1) * (TOKT // QT)):
                        for nt in range(DO // 512):
                            pending.append(("o", tt, nt))
                while pending:
                    emit(pending.popleft())
    nc.finalize()
    return nc


def make_in_maps(x, W_qkv, b_qkv, W_o):
    """Shard full inputs into per-core input maps (core c: batch c//4, group c%4)."""
    x = np.asarray(x, dtype=np.float32)
    W_qkv = np.asarray(W_qkv, dtype=np.float32)
    b_qkv = np.asarray(b_qkv, dtype=np.float32)
    W_o = np.asarray(W_o, dtype=np.float32)
    bf16 = ml_dtypes.bfloat16
    KT = D // 128

    def fold(a):  # [D, C] -> [128, KT, C] bf16
        return np.ascontiguousarray(a.reshape(KT, 128, -1).transpose(1, 0, 2)).astype(
            bf16
        )

    in_maps = []
    for c in range(NCORES):
        b, g = divmod(c, GROUPS)
        cols = slice(CH * g, CH * (g + 1))
        Wv = W_qkv[:, 2 * D : 3 * D][:, cols]
        bv = b_qkv[2 * D : 3 * D][cols]
        Wv_pad = np.zeros((D, NH * VW), dtype=np.float32)
        bv_pad = np.zeros((NH * VW,), dtype=np.float32)
        for h in range(NH):
            Wv_pad[:, VW * h : VW * h + SUB] = Wv[:, SUB * h : SUB * (h + 1)]
            bv_pad[VW * h : VW * h + SUB] = bv[SUB * h : SUB * (h + 1)]
            bv_pad[VW * h + SUB] = 1.0
        m = {
            "xT": fold(x[b].T),
            "wq": fold(W_qkv[:, 0 * D : 1 * D][:, cols]),
            "wk": fold(W_qkv[:, 1 * D : 2 * D][:, cols]),
            "wv": fold(Wv_pad),
            "wvb": bv_pad[None, :].astype(bf16),
            "ones": np.ones((1, 128), dtype=np.float32).astype(bf16),
            "wo": np.ascontiguousarray(W_o[cols, :]).astype(bf16),
            "bq": np.ascontiguousarray(b_qkv[0 * D : 1 * D][cols][:, None]),
            "bk": np.ascontiguousarray(b_qkv[1 * D : 2 * D][cols][:, None]),
        }
        in_maps.append(m)
    return in_maps


_NC = None


def get_nc():
    global _NC
    if _NC is None:
        _NC = build_nc()
    return _NC


def kernel(x, W_qkv, b_qkv, W_o, b_o):
    from concourse import bass_utils

    b_o = np.asarray(b_o, dtype=np.float32)
    in_maps = make_in_maps(x, W_qkv, b_qkv, W_o)
    res = bass_utils.run_bass_kernel_spmd(get_nc(), in_maps, core_ids=list(range(NCORES)))
    out = np.empty((B, N, D), dtype=np.float32)
    for b in range(B):
        acc = res.results[4 * b]["out"].copy()
        for g in range(1, GROUPS):
            acc += res.results[4 * b + g]["out"]
        out[b] = acc + b_o
    return out
